# revision 22
# baseline (speedup 1.0000x reference)
"""Trainium2 Bass kernel for per-pixel dynamic 3D filtering.

    out[b, h, w, o] = sum_k patches[b, h, w, k] * f[b, h, w, k, o]

with patches = im2col(x) over a 3x3 spatial window (zero-padded SAME) and
3 time steps, k ordered (kh, kw, t), K=27, C_out=16, B=8, H=W=192.

Sharding: pure data parallel over batch — core c computes image c.

Default path (v4): f and patches staged host-side in fp16 (the harness
correctness gate is 2e-2; fp16 staging lands ~3.5e-4) — HALVES the dominant
HBM traffic vs fp32. Compute is a hand-written custom-DVE uop program
(DYNF_PAIRMAC_SCAN_ANT) whose 2x_1p perf-mode slot processes a packed fp16
PAIR per cycle: (f0*p0 + f1*p1) folded into a running fp32 prefix — 2 MACs/
cycle, double the fused-1x ceiling. Segments are front-padded 27->28 taps so
every segment-end prefix lands on an odd stream position (pair-reachable);
the scan writes through a last-wins stride-0 out AP, so ONLY the 384
segment-end prefixes land (tiny ends tile), and one dense tensor_sub
recovers all sums. Supertiles are processed in PAIRS (one DMA set / pad /
scan / sub / out-DMA per two row-groups) to amortize per-instruction
issue+drain+sem costs. Measured on 8 axon trn2 cores: 98.0us/iter
(rep-delta), vs 204-208us for the fp32 v2 baseline; floors: DMA 76us
(37.4MB @ ~492GB/s/core), DVE ~78us. fp16 DVE *outputs* crash the exec
unit (NRT status 101) — all DVE outs stay fp32; the final out tensor is
fp32 as required.

Per-core device layout (one image):
  * pixels are mapped to SBUF partitions in 8h x 16w blocks: a "supertile"
    covers 8 image rows x all 192 columns; partition p = dh*16 + dw holds the
    12 consecutive pixels w in [dw*12, dw*12+12).  With this mapping, the
    f-slab of a supertile is EXACTLY a contiguous row-major [128, 5184] slice
    of f, and the out-slab is a contiguous [128, 192] slice of out.
  * im2col of the small x tensor is done host-side and uploaded pre-blocked
    (4MB vs f's 64MB) — generating it on-device would cost more HBM traffic
    than uploading it.

Compute (v2 wide-scan, default): ONE custom DVE instruction per supertile.
The op DYNF_MAC_SCAN_ANT computes scan(ADD, Src0*Src1) — a running prefix
sum of the patch*f products — so every f element is touched exactly once
(vs two passes for the stock tensor_tensor + tensor_reduce pair, the v1
fallback). The key AP trick: the per-pixel stride is 432 = 27*16, so for a
fixed output channel o the whole supertile's (pixel, tap) stream is ONE
affine dim (step 16, count 324); in0 = [P, o:16 step 1, gk:324 step 16]
covers all 12 pixel groups in a single instruction. The prefix is stored
linearly in stream order with a zeroed pad element in front; every
(pixel, o) tap-sum is then prefix[end_i] - prefix[end_i - 27] — uniform
across pixel AND o-row boundaries — recovered by ONE strided tensor_sub.
DVE per supertile: 1 scan (FD 5184) + 1 sub (FD 192); the pad zeroing
rides the idle ACT engine.

Pipelining: f-slab DMA split in two halves on the sync-engine HWDGE ring
(kept as a pure prefetch stream); out-DMAs ride the ACT HWDGE ring so a
sem-waiting store can never stall the next f load; fbufs=3/prefbufs=3/
obufs=6 buffering.

Measured (8 cores concurrent, per-iteration steady state via the
(T(17)-T(1))/16 NEFF-repetition method): ~191-192us vs a ~188-199us
pure-DMA floor for the same 70MB/core traffic — at the HBM memory
roofline. (Stock-op v1: ~310us; 12-scans-per-supertile v2: ~205-230us.)
"""

import os
from contextlib import ExitStack

import numpy as np

# ---- problem constants (hardcoded per contract) ---------------------------
B, T, H, W = 8, 3, 192, 192
K = 3
PAD = K // 2
KK = T * K * K  # 27
CO = 16
N_CORES = 8

# supertile geometry
DH, DW, G = 8, 16, 12  # partitions = DH*DW = 128; per-partition pixels = G
P = DH * DW  # 128
N_ST = H // DH  # 24 supertiles per image
FFREE = G * KK * CO  # 5184 f32 per partition per supertile
PFREE = G * KK  # 324 patch f32 per partition per supertile
OFREE = G * CO  # 192 out f32 per partition per supertile


def _im2col_batch(x: np.ndarray) -> np.ndarray:
    """x: (B, T, H, W) f32 -> patches (B, H, W, 27), k ordered (kh, kw, t)."""
    Bb, Tt, Hh, Ww = x.shape
    xp = np.pad(x, ((0, 0), (0, 0), (PAD, PAD), (PAD, PAD)))
    cols = [
        xp[:, t, i : i + Hh, j : j + Ww]
        for i in range(K)
        for j in range(K)
        for t in range(Tt)
    ]
    return np.stack(cols, axis=-1).astype(np.float32)


# --- v3 (fp16 packed-pair scan) geometry -----------------------------------
KP = KK + 1  # 28: 27 taps + 1 front zero-pad so segment ends land at odd
#              stream positions (pair-granular prefix still extracts sums)
F3FREE = CO * G * KP  # 5376 fp16 per partition per supertile, (o, j, k28)
P3FREE = G * KP  # 336 fp16 patches per partition, (j, k28)
NPAIR = F3FREE // 2  # 2688 pair-prefix values (fp32)
# supertile-PAIR variant: two row-groups u=0,1 fused per instruction set;
# per-partition f stream is (o, u, j, k28) — o-pages of 672 with the two
# groups' patches concatenated, so in1 = [(0, CO), (1, 672)] stays rank-3.
F3FREE2 = 2 * F3FREE  # 10752
P3FREE2 = 2 * P3FREE  # 672
NPAIR2 = F3FREE2 // 2  # 5376

XFREE = T * K * 16  # 144: per-partition per-supertile x-window (wl padded 14->16)


def _xpp_batch(x: np.ndarray) -> np.ndarray:
    """Per-partition x windows: (B,T,H,W) -> (B, N_ST*P, 144), layout
    (t, kh, wl) per partition; value = xp[t, 8s+dh+kh, dw*12+wl], wl<14."""
    xp = np.pad(x, ((0, 0), (0, 0), (PAD, PAD), (PAD, PAD))).astype(np.float32)
    out = np.zeros((x.shape[0], N_ST, DH, DW, T, K, 16), np.float32)
    rows = np.arange(H).reshape(N_ST, DH)
    cols = (np.arange(DW) * G)[:, None] + np.arange(14)[None, :]
    for kh in range(K):
        sub = xp[:, :, rows + kh, :][:, :, :, :, cols]  # (B,T,NST,DH,DW,14)
        out[..., kh, :14] = np.moveaxis(sub, 1, 4)
    return out.reshape(x.shape[0], N_ST * P, XFREE)


def _register_custom_op():
    """Register DYNF_MAC_SCAN_ANT: out = running_sum(in0 * in1) along the free
    stream (inclusive prefix scan of the product). One DVE pass fuses the
    multiply and the k-reduction; segment sums fall out as differences of the
    prefix at segment-end positions."""
    import concourse.dve_ops as dve_ops
    from concourse.dve_spec import AluOp, Spec, Src0, Src1, _has_src1, lower, scan
    from concourse.dve_uop import DveOpSpec

    name = "DYNF_MAC_SCAN_ANT"
    for op in dve_ops.OPS:
        if op.name == name:
            return op

    def _ref(in0, in1, c0, c1, c2):
        prod = np.asarray(in0, np.float32) * np.asarray(in1, np.float32)
        flat = prod.reshape(prod.shape[0], -1)
        return np.cumsum(flat, axis=1, dtype=np.float32).reshape(prod.shape)

    spec = Spec(body=scan(AluOp.ADD, Src0 * Src1), reference=_ref)
    row = dve_ops._CUSTOM_DVE_ROW_BASE + len(dve_ops.OPS)
    assert row < 0x20
    shas = {}
    for ver in ("v3", "v4"):
        s = DveOpSpec(
            name=name, opcode=row, uops=lower(spec, ver=ver), rd1_en=_has_src1(spec)
        )
        shas[ver] = s.sha(ver)
    op = dve_ops.DveOp(name, spec, subdim=False, uops_sha=shas)
    dve_ops.OPS.append(op)
    dve_ops._SUB_OPCODE_FOR_NAME[name] = row
    dve_ops.CUSTOM_DVE_SPECS[name] = spec
    return op


def _register_pair_op():
    """Register DYNF_PAIRMAC_SCAN_ANT: a hand-written uop program computing
    the running sum of Src0*Src1 over PAIRS of packed 16-bit elements, one
    fp32 prefix value written per pair.

    uops (1x fallback, 3 states): seed -> even (accumulate, no write) ->
    odd (accumulate, write) ping-pong; 1 elem/cycle, one write per 2 elems.
    uops_2x (2 states + dummy): seed -> steady; per cycle the packed pair
    (SRC_0, SRC_0_HI) x (SRC_1, SRC_1_HI) is multiplied (2 ALU stages),
    pair-summed, and folded into the CURR_ALU_OUT scan accumulator; ONE
    fp32 write per cycle = 2 MACs/cycle — double the fused-1x rate.

    The instruction's perf_max (byte-36[7:6]) is set to 1 by the emit
    helper so the engine may reach the 2x_1p slot; if the engine's mode
    check declines (e.g. fp32 out), the 1x program still computes the
    same result at half rate."""
    import concourse.dve_ops as dve_ops
    from concourse.dve_spec import AluOp, Spec, Src0, Src1, scan
    from concourse.dve_uop import (
        DISABLE,
        ENABLE,
        AluInp,
        DelayInp,
        DveOpSpec,
        InpSel,
        OutPath,
        OutSel,
        Trigger,
        UopConfig,
    )

    name = "DYNF_PAIRMAC_SCAN_ANT"
    for op in dve_ops.OPS:
        if op.name == name:
            return op

    def _ref(in0, in1, c0, c1, c2):
        a = np.asarray(in0, np.float32).reshape(in0.shape[0], -1)
        b = np.asarray(in1, np.float32).reshape(in1.shape[0], -1)
        c = np.cumsum(a * b, axis=1, dtype=np.float32)
        return c[:, 1::2]

    SRC_DONE = (Trigger.SRC_TENSOR_DONE, Trigger.NONE, Trigger.NONE)
    COUNT_ONCE = (Trigger.COUNT, Trigger.NONE, Trigger.NONE)
    DONE_OR_COUNT = (Trigger.SRC_TENSOR_DONE, Trigger.COUNT, Trigger.NONE)

    def _mk(n_states):
        return [UopConfig() for _ in range(n_states)]

    # ---- 1x program: [seed, even, odd] -----------------------------------
    def _dp_1x(u):
        # lane0=SRC_0, lane1=SRC_1; blk0: m = f*p; blk1: s += m; rest bypass
        u.enable_input(InpSel.SRC_0, 1)
        u.enable_input(InpSel.SRC_1, 2)
        u.datapath_config[0].enable_alu(
            AluOp.MULTIPLY, AluInp.PREV_DELAY_0, AluInp.PREV_DELAY_1
        )
        u.datapath_config[1].enable_alu(
            AluOp.ADD, AluInp.CURR_ALU_OUT, AluInp.PREV_ALU_OUT
        )
        for b in range(2, 8):
            u.datapath_config[b].pass_through_alu()
        u.require_inp0 = 1
        u.require_inp1 = 1

    seed1, even1, odd1 = _mk(3)
    # seed: blk1 out-flop <- 0 via ZERO on lane4; consumes nothing, no write
    seed1.enable_input(InpSel.ZERO, 5)
    for b in range(0, 1):
        seed1.datapath_config[b].pass_through_delay(4)
    seed1.datapath_config[1].enable_alu(AluOp.BYPASS, AluInp.PREV_DELAY_4)
    seed1.trigger = COUNT_ONCE
    seed1.repeat_count = 1
    seed1.next_uop = (1, 0, 0)

    _dp_1x(even1)
    even1.trigger = DONE_OR_COUNT
    even1.repeat_count = 1
    even1.next_uop = (0, 2, 0)

    _dp_1x(odd1)
    odd1.enable_output(OutSel.ALU_OUT, OutPath.WR0_LO)
    odd1.trigger = DONE_OR_COUNT
    odd1.repeat_count = 1
    odd1.next_uop = (0, 1, 0)

    uops_1x = [seed1, even1, odd1]

    # ---- 2x program: [seed, steady, dummy] -------------------------------
    seed2, steady2, dummy2 = _mk(3)
    # seed: blk3 out-flop <- 0
    seed2.enable_input(InpSel.ZERO, 5)
    for b in range(0, 3):
        seed2.datapath_config[b].pass_through_delay(4)
    seed2.datapath_config[3].enable_alu(AluOp.BYPASS, AluInp.PREV_DELAY_4)
    seed2.trigger = COUNT_ONCE
    seed2.repeat_count = 1
    seed2.next_uop = (1, 0, 0)

    # steady: blk0 m0 = lo(f)*lo(p); blk1 m1 = hi(f)*hi(p), capture m0;
    # blk2 u = m1 + m0; blk3 s += u (scan feedback); blk4-7 bypass; write s.
    steady2.enable_input(InpSel.SRC_0, 1)
    steady2.enable_input(InpSel.SRC_1, 2)
    steady2.enable_input(InpSel.SRC_0_HI, 3)
    steady2.enable_input(InpSel.SRC_1_HI, 4)
    dp = steady2.datapath_config
    dp[0].enable_alu(AluOp.MULTIPLY, AluInp.PREV_DELAY_0, AluInp.PREV_DELAY_1)
    dp[0].pass_through_delay(2, 3)
    dp[1].enable_alu(AluOp.MULTIPLY, AluInp.PREV_DELAY_2, AluInp.PREV_DELAY_3)
    dp[1].enable_delay_from_src(DelayInp.PREV_ALU_OUT, 0)
    dp[2].enable_alu(AluOp.ADD, AluInp.PREV_ALU_OUT, AluInp.PREV_DELAY_0)
    dp[3].enable_alu(AluOp.ADD, AluInp.CURR_ALU_OUT, AluInp.PREV_ALU_OUT)
    for b in range(4, 8):
        dp[b].pass_through_alu()
    steady2.enable_output(OutSel.ALU_OUT, OutPath.WR0_LO)
    steady2.require_inp0 = 1
    steady2.require_inp1 = 1
    steady2.trigger = SRC_DONE
    steady2.next_uop = (0, 0, 0)

    uops_2x = [seed2, steady2, dummy2]

    spec = Spec(body=scan(AluOp.ADD, Src0 * Src1), reference=_ref)
    row = dve_ops._CUSTOM_DVE_ROW_BASE + len(dve_ops.OPS)
    assert row < 0x20
    shas = {}
    for ver in ("v3", "v4"):
        s = DveOpSpec(
            name=name,
            opcode=row,
            uops=uops_1x,
            uops_2x=uops_2x,
            perf_max=1,
            rd1_en=True,
        )
        s.validate(ver)
        shas[ver] = s.sha(ver)
        dve_ops._COMPILE_CACHE[(name, ver)] = s
    op = dve_ops.DveOp(name, spec, subdim=False, uops_sha=shas)
    dve_ops.OPS.append(op)
    dve_ops._SUB_OPCODE_FOR_NAME[name] = row
    dve_ops.CUSTOM_DVE_SPECS[name] = spec
    return op


def _emit_pair_scan(nc, op, out, in0, in1):
    """Emit one DYNF_PAIRMAC_SCAN instruction with perf_max=1 (2x-reachable).
    Mirrors bass.BassVectorEngine._custom_dve but without the out/in shape
    coupling (out carries ONE fp32 per input pair) and with byte-36[7:6] set."""
    from concourse import bass_isa, mybir
    from concourse.dve_ops import get_dve_sub_opcode

    eng = nc.vector
    if op.name not in nc.m.ant_custom_dve_ops:
        nc.m.ant_custom_dve_ops = sorted({*nc.m.ant_custom_dve_ops, op.name})
    shape = bass_isa.CustomDveShape.STT  # in1 has 2 free dims
    isa_opcode = nc.isa.Opcode[
        f"NEURON_ISA_TPB_OPCODE_CUSTOM_DVE_ANT_{shape.slot()}"
    ].value
    imm = mybir.ImmediateValue(dtype=mybir.dt.float32, value=0.0)
    inst = bass_isa.InstCustomDveAnt(
        name=nc.get_next_instruction_name(),
        op_name=op.name,
        rd1_en=True,
        subdim=0,
        imm2=0.0,
        shape=shape,
        row=get_dve_sub_opcode(op.name),
        isa_opcode=isa_opcode,
        ins=[
            eng.lower_ap(in0, for_isa=True, opt=True),
            eng.lower_ap(in1, for_isa=True, opt=True),
            imm,
            mybir.ImmediateValue(dtype=mybir.dt.float32, value=0.0),
        ],
        outs=[eng.lower_ap(out, for_isa=True, opt=True)],
    )
    inst.perf_max = 1
    return eng.add_instruction(inst)


def _build_program_v3(reps: int = 1, mode: str = "full"):
    """v3: fp16-staged f/p (half the HBM traffic of v2) + packed-pair custom
    DVE scan (2 MACs/cycle — half the DVE time of v2's wide scan).

    Per-partition per-supertile stream, (o, j, k28)-major with k28 = zero pad
    followed by the 27 taps: segment (o, j) occupies elements
    [o*336 + j*28, .. + 27], whose cumulative end position is ODD, so the
    once-per-pair prefix stream (written at offset 1 over a zeroed pad slot)
    contains every segment-end prefix; sums fall out of ONE strided
    tensor_sub at pair stride 14 exactly as in v2."""
    import concourse.bacc as bacc
    import concourse.tile as tile
    from concourse import mybir

    f32 = mybir.dt.float32
    f16 = mybir.dt.float16
    pair_op = _register_pair_op()

    nc = bacc.Bacc("TRN2", debug=False, enable_asserts=False)

    out16 = os.environ.get("DYNF_OUT16", "0") == "1"
    odt_out = f16 if out16 else f32
    f_ap = nc.dram_tensor("f_in", (N_ST * P, F3FREE), f16, kind="ExternalInput").ap()
    p_ap = nc.dram_tensor("p_in", (N_ST * P, P3FREE), f16, kind="ExternalInput").ap()
    o_ap = nc.dram_tensor(
        "o_out", (N_ST * P, OFREE), odt_out, kind="ExternalOutput"
    ).ap()

    fbufs = int(os.environ.get("DYNF_FBUFS", "3"))
    prefbufs = int(os.environ.get("DYNF_PREFBUFS", "3"))
    obufs = int(os.environ.get("DYNF_OBUFS", "6"))
    alloc_mode = os.environ.get("DYNF_POOL_ALLOC", "stack")

    with tile.TileContext(nc, pool_alloc_mode=alloc_mode) as tc, ExitStack() as ctx:
        fpool = ctx.enter_context(tc.tile_pool(name="fpool", bufs=fbufs))
        ppool = ctx.enter_context(tc.tile_pool(name="ppool", bufs=3))
        prefpool = ctx.enter_context(tc.tile_pool(name="prefpool", bufs=prefbufs))
        opool = ctx.enter_context(tc.tile_pool(name="opool", bufs=obufs))

        zpool = ctx.enter_context(tc.tile_pool(name="zpool", bufs=1))
        zerot = zpool.tile([P, 1], f32)
        nc.vector.memset(zerot[:], 0.0)

        if mode in ("dve", "dve16"):
            # pure DVE throughput probe: resident f/p tiles, scans only.
            # dve16: fp16 prefix out — numerically useless, measures whether
            # a 16-bit out dtype is what gates the 2x perf mode.
            ft0 = fpool.tile([P, F3FREE], f16)
            nc.sync.dma_start(ft0[:], f_ap[0:P, :])
            pt0 = ppool.tile([P, P3FREE], f16, tag="pt")
            nc.sync.dma_start(pt0[:], p_ap[0:P, :])
            odt = f16 if mode == "dve16" else f32
            APc = None
            zerot16 = zpool.tile([P, 1], f16)
            nc.vector.memset(zerot16[:], 0.0)
            for _ in range(reps):
                for s in range(N_ST):
                    rows = slice(s * P, (s + 1) * P)
                    pref = prefpool.tile([P, NPAIR + 1], odt)
                    nc.scalar.copy(
                        pref[:, 0:1], zerot[:] if odt == f32 else zerot16[:]
                    )
                    APc = type(ft0[:])
                    fa, pa, pra = ft0[:], pt0[:], pref[:]
                    in0 = APc(fa.tensor, fa.offset, [list(fa.ap[0]), [1, F3FREE]])
                    in1 = APc(
                        pa.tensor, pa.offset, [list(pa.ap[0]), [0, CO], [1, P3FREE]]
                    )
                    outp = APc(
                        pra.tensor, pra.offset + 1, [list(pra.ap[0]), [1, NPAIR]]
                    )
                    _emit_pair_scan(nc, pair_op, outp, in0, in1)
                    ot = opool.tile([P, OFREE], f32)
                    nc.vector.tensor_copy(ot[:], pref[:, 1 : OFREE + 1])
                    nc.scalar.dma_start(o_ap[rows, :], ot[:])

        for _ in range(reps if mode not in ("dve", "dve16") else 0):
            for s in range(N_ST):
                rows = slice(s * P, (s + 1) * P)
                ft = fpool.tile([P, F3FREE], f16)
                nsplit = int(os.environ.get("DYNF_SPLIT", "2"))
                hw_elems = F3FREE // nsplit
                for h in range(nsplit):
                    nc.sync.dma_start(
                        ft[:, h * hw_elems : (h + 1) * hw_elems],
                        f_ap[rows, h * hw_elems : (h + 1) * hw_elems],
                    )
                pt = ppool.tile([P, P3FREE], f16, tag="pt")
                nc.sync.dma_start(pt[:], p_ap[rows, :])

                if mode == "dma":
                    ot = opool.tile([P, OFREE], f32)
                    nc.vector.memset(ot[:], 0.0)
                    nc.scalar.dma_start(o_ap[rows, :], ot[:])
                    continue

                pref = prefpool.tile([P, NPAIR + 1], f32)
                nc.scalar.copy(pref[:, 0:1], zerot[:])

                APc = type(ft[:])
                fa, pa, pra = ft[:], pt[:], pref[:]
                in0 = APc(fa.tensor, fa.offset, [list(fa.ap[0]), [1, F3FREE]])
                in1 = APc(
                    pa.tensor, pa.offset, [list(pa.ap[0]), [0, CO], [1, P3FREE]]
                )
                outp = APc(pra.tensor, pra.offset + 1, [list(pra.ap[0]), [1, NPAIR]])
                _emit_pair_scan(nc, pair_op, outp, in0, in1)

                if mode == "scan":
                    nc.scalar.dma_start(o_ap[rows, :], pref[:, :OFREE])
                    continue

                # segment sums: pref is the pair-prefix stream (offset 1 over
                # the zero pad); end(o,j) at pair o*168+j*14+13 -> +1 = +14,
                # prev end 14 earlier. sub writes (o outer, j minor) into the
                # (j, o) out layout via strides.
                ot = opool.tile([P, OFREE], odt_out)
                oa = ot[:]
                sub_out = APc(oa.tensor, oa.offset, [list(oa.ap[0]), [1, CO], [CO, G]])
                PAIRS_O = NPAIR // CO  # 168
                e1 = APc(
                    pra.tensor,
                    pra.offset + KP // 2,
                    [list(pra.ap[0]), [PAIRS_O, CO], [KP // 2, G]],
                )
                e0 = APc(
                    pra.tensor,
                    pra.offset,
                    [list(pra.ap[0]), [PAIRS_O, CO], [KP // 2, G]],
                )
                nc.vector.tensor_sub(sub_out, e1, e0)
                nc.scalar.dma_start(o_ap[rows, :], ot[:])

    nc.compile()
    return nc


def _build_program_v4(reps: int = 1, mode: str = "full"):
    """v4: v3's packed-pair 2x scan with supertiles fused in PAIRS — one
    f-DMA / pad-zero / scan / sub / out-DMA per TWO image row-groups, halving
    per-instruction startup+drain+sem overhead. Stream per partition is
    (o, u, j, k28) with u the row-group: o-pages of 672 keep in1 rank-3, and
    the global-prefix difference trick is unchanged (ends at odd positions,
    pair stride 14)."""
    import concourse.bacc as bacc
    import concourse.tile as tile
    from concourse import mybir

    f32 = mybir.dt.float32
    f16 = mybir.dt.float16
    pair_op = _register_pair_op()
    NQ = N_ST // 2  # 12 supertile pairs

    nc = bacc.Bacc("TRN2", debug=False, enable_asserts=False)

    out16 = os.environ.get("DYNF_OUT16", "0") == "1"
    odt_out = f16 if out16 else f32
    f_ap = nc.dram_tensor("f_in", (NQ * P, F3FREE2), f16, kind="ExternalInput").ap()
    p_ap = nc.dram_tensor("p_in", (NQ * P, P3FREE2), f16, kind="ExternalInput").ap()
    o_ap = nc.dram_tensor(
        "o_out", (N_ST * P, OFREE), odt_out, kind="ExternalOutput"
    ).ap()

    ends_default = os.environ.get("DYNF_ENDS", "1") == "1"
    fbufs = int(os.environ.get("DYNF_FBUFS", "4" if ends_default else "3"))
    prefbufs = int(os.environ.get("DYNF_PREFBUFS", "3"))
    obufs = int(os.environ.get("DYNF_OBUFS", "4"))
    alloc_mode = os.environ.get("DYNF_POOL_ALLOC", "stack")
    pad_eng = os.environ.get("DYNF_PAD_ENGINE", "vector")

    with tile.TileContext(nc, pool_alloc_mode=alloc_mode) as tc, ExitStack() as ctx:
        fpool = ctx.enter_context(tc.tile_pool(name="fpool", bufs=fbufs))
        ppool = ctx.enter_context(tc.tile_pool(name="ppool", bufs=3))
        prefpool = ctx.enter_context(tc.tile_pool(name="prefpool", bufs=prefbufs))
        opool = ctx.enter_context(tc.tile_pool(name="opool", bufs=obufs))

        zpool = ctx.enter_context(tc.tile_pool(name="zpool", bufs=1))
        zerot = zpool.tile([P, 1], f32)
        nc.vector.memset(zerot[:], 0.0)

        for _ in range(reps):
            for q in range(NQ):
                rows = slice(q * P, (q + 1) * P)
                ft = fpool.tile([P, F3FREE2], f16)
                nsplit = int(os.environ.get("DYNF_SPLIT", "4"))
                hw_elems = F3FREE2 // nsplit
                for h in range(nsplit):
                    nc.sync.dma_start(
                        ft[:, h * hw_elems : (h + 1) * hw_elems],
                        f_ap[rows, h * hw_elems : (h + 1) * hw_elems],
                    )
                pt = ppool.tile([P, P3FREE2], f16, tag="pt")
                nc.sync.dma_start(pt[:], p_ap[rows, :])

                if mode == "dma":
                    # DMA floor probe: consume ft/pt minimally so the loads
                    # aren't dead-code-eliminated, then write out.
                    ot = opool.tile([P, 2 * OFREE], odt_out)
                    nc.vector.tensor_copy(ot[:, 0:1], ft[:, 0:1])
                    nc.vector.tensor_copy(ot[:, 1:2], pt[:, 0:1])
                    dst = type(ft[:])(
                        o_ap.tensor,
                        o_ap.offset + 2 * q * P * OFREE,
                        [[OFREE, P], [P * OFREE, 2], [1, OFREE]],
                    )
                    nc.scalar.dma_start(dst, ot[:])
                    continue

                ends_mode = os.environ.get("DYNF_ENDS", "1") == "1"
                APc = type(ft[:])
                NSEG = 2 * OFREE  # 384 segment ends per pair-unit
                if ends_mode:
                    # last-wins out AP: the scan streams 5376 pair-prefix
                    # writes through [[1,384],[0,14]] — only each 14th (the
                    # segment-end prefix) survives, landing densely in a tiny
                    # [P, 385] ends tile (slot 0 = zero pad). HW-verified
                    # trick (v2 ends_direct).
                    pref = prefpool.tile([P, NSEG + 1], f32)
                else:
                    pref = prefpool.tile([P, NPAIR2 + 1], f32)
                if pad_eng == "scalar":
                    nc.scalar.copy(pref[:, 0:1], zerot[:])
                else:
                    nc.vector.memset(pref[:, 0:1], 0.0)

                fa, pa, pra = ft[:], pt[:], pref[:]
                in0 = APc(fa.tensor, fa.offset, [list(fa.ap[0]), [1, F3FREE2]])
                in1 = APc(
                    pa.tensor, pa.offset, [list(pa.ap[0]), [0, CO], [1, P3FREE2]]
                )
                if ends_mode:
                    outp = APc(
                        pra.tensor,
                        pra.offset + 1,
                        [list(pra.ap[0]), [1, NSEG], [0, KP // 2]],
                    )
                else:
                    outp = APc(
                        pra.tensor, pra.offset + 1, [list(pra.ap[0]), [1, NPAIR2]]
                    )
                _emit_pair_scan(nc, pair_op, outp, in0, in1)

                if mode == "scan":
                    nc.scalar.dma_start(
                        o_ap[slice(2 * q * P, (2 * q + 1) * P), :],
                        pref[:, :OFREE],
                    )
                    continue

                ot = opool.tile([P, 2 * OFREE], odt_out)
                oa = ot[:]
                sub_out = APc(
                    oa.tensor,
                    oa.offset,
                    [list(oa.ap[0]), [1, CO], [OFREE, 2], [CO, G]],
                )
                if ends_mode:
                    # ends[1 + o*24 + u*12 + j]; prev end is index-1 flat
                    e1 = APc(
                        pra.tensor,
                        pra.offset + 1,
                        [list(pra.ap[0]), [2 * G, CO], [G, 2], [1, G]],
                    )
                    e0 = APc(
                        pra.tensor,
                        pra.offset,
                        [list(pra.ap[0]), [2 * G, CO], [G, 2], [1, G]],
                    )
                else:
                    # ends at pair o*336 + u*168 + j*14 + 13 (+1 pad offset)
                    e1 = APc(
                        pra.tensor,
                        pra.offset + KP // 2,
                        [list(pra.ap[0]), [NPAIR2 // CO, CO], [168, 2], [KP // 2, G]],
                    )
                    e0 = APc(
                        pra.tensor,
                        pra.offset,
                        [list(pra.ap[0]), [NPAIR2 // CO, CO], [168, 2], [KP // 2, G]],
                    )
                nc.vector.tensor_sub(sub_out, e1, e0)

                o_t = o_ap
                dst = APc(
                    o_t.tensor,
                    o_t.offset + 2 * q * P * OFREE,
                    [[OFREE, P], [P * OFREE, 2], [1, OFREE]],
                )
                nc.scalar.dma_start(dst, ot[:])

    nc.compile()
    return nc


def _build_program_v2(reps: int = 1, mode: str = "full"):
    """v2: fused multiply+scan custom DVE op — one DVE pass over f instead of
    two (tensor_tensor mult + tensor_reduce).

    mode: "full" | "dma" (no compute) | "scan" (no extraction) — diagnostics."""
    import concourse.bacc as bacc
    import concourse.tile as tile
    from concourse import mybir

    f32 = mybir.dt.float32
    mac_op = _register_custom_op()
    patch_mode = os.environ.get("DYNF_PATCH_MODE", "packed")

    nc = bacc.Bacc("TRN2", debug=False, enable_asserts=False)

    f_ap = nc.dram_tensor("f_in", (N_ST * P, FFREE), f32, kind="ExternalInput").ap()
    if patch_mode == "expand":
        p_ap = nc.dram_tensor(
            "p_in", (N_ST * P, XFREE), f32, kind="ExternalInput"
        ).ap()
    else:
        p_ap = nc.dram_tensor(
            "p_in", (N_ST * P, PFREE), f32, kind="ExternalInput"
        ).ap()
    o_ap = nc.dram_tensor("o_out", (N_ST * P, OFREE), f32, kind="ExternalOutput").ap()

    fbufs = int(os.environ.get("DYNF_FBUFS", "3"))
    prefbufs = int(os.environ.get("DYNF_PREFBUFS", "3"))
    obufs = int(os.environ.get("DYNF_OBUFS", "6"))
    # default: extraction on DVE. gpsimd-extraction measured faster once but
    # produced NRT_EXEC_UNIT_UNRECOVERABLE device crashes when combined with
    # split f-DMAs — not worth the risk.
    ext_eng = os.environ.get("DYNF_EXT_ENGINE", "vector")
    alloc_mode = os.environ.get("DYNF_POOL_ALLOC", "stack")

    with tile.TileContext(nc, pool_alloc_mode=alloc_mode) as tc, ExitStack() as ctx:
        fpool = ctx.enter_context(tc.tile_pool(name="fpool", bufs=fbufs))
        ppool = ctx.enter_context(tc.tile_pool(name="ppool", bufs=3))
        prefpool = ctx.enter_context(tc.tile_pool(name="prefpool", bufs=prefbufs))
        opool = ctx.enter_context(tc.tile_pool(name="opool", bufs=obufs))

        zpool = ctx.enter_context(tc.tile_pool(name="zpool", bufs=1))
        zerot = zpool.tile([P, 1], f32)
        nc.vector.memset(zerot[:], 0.0)

        if mode == "dve":
            # pure DVE throughput probe: one resident f/p tile, all scans
            ft0 = fpool.tile([P, FFREE], f32)
            nc.sync.dma_start(ft0[:], f_ap[0:P, :])
            pt0 = ppool.tile([P, PFREE], f32, tag="pt")
            nc.sync.dma_start(pt0[:], p_ap[0:P, :])
            for _ in range(reps):
                for s in range(N_ST):
                    rows = slice(s * P, (s + 1) * P)
                    pref = prefpool.tile([P, FFREE], f32)
                    for g in range(G):
                        f_ok = ft0[:, g * KK * CO : (g + 1) * KK * CO].rearrange(
                            "p (k o) -> p o k", k=KK, o=CO
                        )
                        p_ok = (
                            pt0[:, g * KK : (g + 1) * KK]
                            .unsqueeze(1)
                            .broadcast_to([P, CO, KK])
                        )
                        pr_ok = pref[
                            :, g * KK * CO : (g + 1) * KK * CO
                        ].rearrange("p (o k) -> p o k", o=CO, k=KK)
                        nc.vector._custom_dve(
                            mac_op, out=pr_ok, in0=f_ok, in1=p_ok
                        )
                    nc.scalar.dma_start(o_ap[rows, :], pref[:, :OFREE])
            nc.compile()
            return nc

        for _ in range(reps):
            for s in range(N_ST):
                rows = slice(s * P, (s + 1) * P)
                ft = fpool.tile([P, FFREE], f32)
                nsplit = int(os.environ.get("DYNF_SPLIT", "2"))
                hw_elems = FFREE // nsplit
                for h in range(nsplit):
                    nc.sync.dma_start(
                        ft[:, h * hw_elems : (h + 1) * hw_elems],
                        f_ap[rows, h * hw_elems : (h + 1) * hw_elems],
                    )
                if patch_mode == "expand":
                    xt = ppool.tile([P, XFREE], f32, tag="xt")
                    nc.sync.dma_start(xt[:], p_ap[rows, :])
                    # expand windows -> patches on GPSIMD (idle engine):
                    # pt[g, kh, kw, t] = xt[t, kh, g+kw]
                    pt = ppool.tile([P, PFREE], f32, tag="pt")
                    pt5 = pt[:].rearrange(
                        "p (g kh kw t) -> p kh g kw t", g=G, kh=K, kw=K, t=T
                    )
                    xta = xt[:]
                    APc = type(xta)
                    exp_name = os.environ.get("DYNF_EXPAND_ENGINE", "scalar")
                    for kh in range(K):
                        src = APc(
                            xta.tensor,
                            xta.offset + kh * 16,
                            [list(xta.ap[0]), [1, G], [1, K], [K * 16, T]],
                        )
                        if exp_name == "scalar":
                            nc.scalar.copy(pt5[:, kh], src)
                        elif exp_name == "gpsimd":
                            nc.gpsimd.tensor_copy(pt5[:, kh], src)
                        else:
                            nc.vector.tensor_copy(pt5[:, kh], src)
                else:
                    pt = ppool.tile([P, PFREE], f32, tag="pt")
                    if os.environ.get("DYNF_PT_ENGINE", "sync") == "scalar":
                        nc.scalar.dma_start(pt[:], p_ap[rows, :])
                    else:
                        nc.sync.dma_start(pt[:], p_ap[rows, :])

                if mode == "dma":
                    nc.scalar.dma_start(o_ap[rows, :], ft[:, :OFREE])
                    continue

                if os.environ.get("DYNF_SCAN_WIDE", "1") == "1":
                    # ONE scan per supertile: for fixed o, addr(g,k) =
                    # (g*27+k)*16 + o is a single affine dim (432 == 27*16),
                    # so in0 = [P, o:16 step 1, gk:324 step 16] covers all 12
                    # pixel groups. Prefix stored linearly in stream order
                    # (offset 1; [0] is a pad so the i=0 difference stays
                    # in-tile); segment ends sit exactly 27 apart, so ONE
                    # tensor_sub recovers every segment sum — the -27
                    # neighbour is correct even across o-row boundaries.
                    pref = prefpool.tile([P, FFREE + 1], f32)
                    # zero the pad so the i=0 difference is E0 - 0. On DVE by
                    # default: an ACT-side copy would sit on the ACT queue
                    # ahead of out-DMAs carrying a pref-slot dependency.
                    if os.environ.get("DYNF_PAD_ENGINE", "scalar") == "scalar":
                        nc.scalar.copy(pref[:, 0:1], zerot[:])
                    else:
                        nc.vector.memset(pref[:, 0:1], 0.0)
                    APc = type(ft[:])
                    fa, pa, pra = ft[:], pt[:], pref[:]
                    GK = G * KK  # 324
                    in0 = APc(
                        fa.tensor, fa.offset, [list(fa.ap[0]), [1, CO], [CO, GK]]
                    )
                    in1 = APc(
                        pa.tensor, pa.offset, [list(pa.ap[0]), [0, CO], [1, GK]]
                    )
                    outp = APc(
                        pra.tensor,
                        pra.offset + 1,
                        [list(pra.ap[0]), [GK, CO], [1, GK]],
                    )
                    nc.vector._custom_dve(mac_op, out=outp, in0=in0, in1=in1)

                    if mode == "scan":
                        nc.scalar.dma_start(o_ap[rows, :], pref[:, :OFREE])
                        continue

                    ot = opool.tile([P, OFREE], f32)
                    oa = ot[:]
                    sub_out = APc(
                        oa.tensor, oa.offset, [list(oa.ap[0]), [1, CO], [CO, G]]
                    )
                    e1 = APc(
                        pra.tensor,
                        pra.offset + KK,
                        [list(pra.ap[0]), [GK, CO], [KK, G]],
                    )
                    e0 = APc(
                        pra.tensor, pra.offset, [list(pra.ap[0]), [GK, CO], [KK, G]]
                    )
                    eng = nc.gpsimd if ext_eng == "gpsimd" else nc.vector
                    eng.tensor_sub(sub_out, e1, e0)
                    if os.environ.get("DYNF_OUT_ENGINE", "scalar") == "sync":
                        nc.sync.dma_start(o_ap[rows, :], ot[:])
                    else:
                        nc.scalar.dma_start(o_ap[rows, :], ot[:])
                    continue

                ends_direct = os.environ.get("DYNF_ENDS_DIRECT", "0") == "1"
                if ends_direct:
                    # scans write through a step-0 (last-wins) out AP: only
                    # each segment's final prefix value survives, landing in a
                    # compact [P, G*CO] ends tile. No prefix buffer at all.
                    endst = prefpool.tile([P, OFREE], f32)
                    APc = type(ft[:])
                    ea = endst[:]
                    for g in range(G):
                        f_ok = ft[:, g * KK * CO : (g + 1) * KK * CO].rearrange(
                            "p (k o) -> p o k", k=KK, o=CO
                        )
                        p_ok = (
                            pt[:, g * KK : (g + 1) * KK]
                            .unsqueeze(1)
                            .broadcast_to([P, CO, KK])
                        )
                        e_ok = APc(
                            ea.tensor,
                            ea.offset + g * CO,
                            [list(ea.ap[0]), [1, CO], [0, KK]],
                        )
                        nc.vector._custom_dve(mac_op, out=e_ok, in0=f_ok, in1=p_ok)
                    ends = ea.rearrange("p (g o) -> p g o", g=G, o=CO)
                    if mode == "scan":
                        nc.scalar.dma_start(o_ap[rows, :], endst[:])
                        continue
                else:
                    # prefix sums of products, (o, k)-major per pixel slot
                    pref = prefpool.tile([P, FFREE], f32)
                    for g in range(G):
                        f_ok = ft[:, g * KK * CO : (g + 1) * KK * CO].rearrange(
                            "p (k o) -> p o k", k=KK, o=CO
                        )
                        p_ok = (
                            pt[:, g * KK : (g + 1) * KK]
                            .unsqueeze(1)
                            .broadcast_to([P, CO, KK])
                        )
                        pr_ok = pref[
                            :, g * KK * CO : (g + 1) * KK * CO
                        ].rearrange("p (o k) -> p o k", o=CO, k=KK)
                        nc.vector._custom_dve(mac_op, out=pr_ok, in0=f_ok, in1=p_ok)

                    if mode == "scan":
                        nc.scalar.dma_start(o_ap[rows, :], pref[:, :OFREE])
                        continue

                    pref4 = pref[:].rearrange(
                        "p (g o k) -> p g o k", g=G, o=CO, k=KK
                    )
                    ends = pref4[:, :, :, KK - 1 : KK].squeeze(3)  # [P, G, CO]

                # segment sums = differences of prefix at k = KK-1 positions
                ot = opool.tile([P, OFREE], f32)
                ot3 = ot[:].rearrange("p (g o) -> p g o", g=G, o=CO)
                eng = nc.gpsimd if ext_eng == "gpsimd" else nc.vector
                # the 1-input o=0 copy rides the otherwise-idle ACT engine
                nc.scalar.copy(ot3[:, :, 0:1], ends[:, :, 0:1])
                eng.tensor_sub(
                    ot3[:, :, 1:CO], ends[:, :, 1:CO], ends[:, :, 0 : CO - 1]
                )

                # out-DMA on the ACT HWDGE ring: keeps the sync-engine ring a
                # pure f/p prefetch stream (a sem-waiting out-DMA on the same
                # FIFO would stall the next supertile's f load).
                if mode == "ext":
                    nc.scalar.dma_start(o_ap[rows, :], ft[:, :OFREE])
                else:
                    nc.scalar.dma_start(o_ap[rows, :], ot[:])

    nc.compile()
    return nc


def _build_program(reps: int = 1):
    """Build the Bass/Tile program once; returns nc.

    reps > 1 repeats the whole per-image computation (benchmark variant:
    dispatch overhead cancels in (T(reps) - T(1)) / (reps - 1))."""
    import concourse.bacc as bacc
    import concourse.tile as tile
    from concourse import mybir

    f32 = mybir.dt.float32

    nc = bacc.Bacc("TRN2", debug=False, enable_asserts=False)

    f_ap = nc.dram_tensor("f_in", (N_ST * P, FFREE), f32, kind="ExternalInput").ap()
    p_ap = nc.dram_tensor("p_in", (N_ST * P, PFREE), f32, kind="ExternalInput").ap()
    o_ap = nc.dram_tensor("o_out", (N_ST * P, OFREE), f32, kind="ExternalOutput").ap()

    with tile.TileContext(nc) as tc, ExitStack() as ctx:
        fpool = ctx.enter_context(tc.tile_pool(name="fpool", bufs=3))
        ppool = ctx.enter_context(tc.tile_pool(name="ppool", bufs=3))
        prodpool = ctx.enter_context(tc.tile_pool(name="prodpool", bufs=2))
        opool = ctx.enter_context(tc.tile_pool(name="opool", bufs=3))

        for _ in range(reps):
            for s in range(N_ST):
                rows = slice(s * P, (s + 1) * P)
                ft = fpool.tile([P, FFREE], f32)
                nc.sync.dma_start(ft[:], f_ap[rows, :])
                pt = ppool.tile([P, PFREE], f32)
                nc.sync.dma_start(pt[:], p_ap[rows, :])

                # products: [128, (g, k, o)] = f * patches (broadcast on o)
                prod = prodpool.tile([P, FFREE], f32)
                f_gko = ft[:].rearrange("p (g k o) -> p g k o", g=G, k=KK, o=CO)
                p_gk1 = (
                    pt[:]
                    .rearrange("p (g k) -> p g k", g=G, k=KK)
                    .unsqueeze(3)
                    .broadcast_to([P, G, KK, CO])
                )
                prod_gko = prod[:].rearrange(
                    "p (g k o) -> p g k o", g=G, k=KK, o=CO
                )
                nc.vector.tensor_tensor(prod_gko, f_gko, p_gk1, mybir.AluOpType.mult)

                # reduce over k (innermost axis of the presented AP)
                ot = opool.tile([P, OFREE], f32)
                prod_gok = prod[:].rearrange("p (g k o) -> p g o k", g=G, k=KK, o=CO)
                ot_go = ot[:].rearrange("p (g o) -> p g o", g=G, o=CO)
                nc.vector.tensor_reduce(
                    ot_go, prod_gok, mybir.AxisListType.X, mybir.AluOpType.add
                )

                nc.sync.dma_start(o_ap[rows, :], ot[:])

    nc.compile()
    return nc


_NC_CACHE = None

# test harness introspection: last BassKernelResults (exec_time_ns when traced)
LAST_RESULTS = None


def build_program(reps: int = 1):
    ver = os.environ.get("DYNF_KERNEL_VERSION", "4")
    if ver == "4":
        try:
            return _build_program_v4(reps, mode=os.environ.get("DYNF_MODE", "full"))
        except Exception:
            os.environ["DYNF_KERNEL_VERSION"] = "3"
            ver = "3"
    if ver == "3":
        try:
            return _build_program_v3(reps, mode=os.environ.get("DYNF_MODE", "full"))
        except Exception:
            os.environ["DYNF_KERNEL_VERSION"] = "2"
            ver = "2"
    if ver == "2":
        try:
            return _build_program_v2(reps)
        except Exception:
            # custom-DVE registration/lowering failed (e.g. concourse drift):
            # fall back to the stock-op kernel (slower but correct). Flag the
            # fallback so prepare_in_maps stages the matching p_in layout.
            os.environ["DYNF_KERNEL_VERSION"] = "1"
            os.environ.pop("DYNF_PATCH_MODE", None)
    return _build_program(reps)


def _get_nc():
    global _NC_CACHE
    if _NC_CACHE is None:
        _NC_CACHE = build_program(1)
    return _NC_CACHE


def prepare_in_maps(x: np.ndarray, f: np.ndarray) -> list[dict]:
    """Host-side staging: per-core {f_in, p_in} in the device DRAM layouts."""
    x = np.asarray(x, dtype=np.float32)
    f = np.asarray(f, dtype=np.float32)
    assert x.shape == (B, T, H, W) and f.shape == (B, H, W, KK, CO)

    if os.environ.get("DYNF_KERNEL_VERSION", "4") == "4":
        # v4 fp16 paired layouts: f (q,dh,dw | o,u,j,k28), p (q,dh,dw | u,j,k28)
        patches = _im2col_batch(x)
        NQ = N_ST // 2
        fr = f.reshape(B, NQ, 2, DH, DW, G, KK, CO)
        f4 = np.zeros((B, NQ, DH, DW, CO, 2, G, KP), np.float16)
        f4[..., 1:] = fr.transpose(0, 1, 3, 4, 7, 2, 5, 6)
        f_blk = f4.reshape(B, NQ * P, F3FREE2)
        pr = patches.reshape(B, NQ, 2, DH, DW, G, KK)
        p4 = np.zeros((B, NQ, DH, DW, 2, G, KP), np.float16)
        p4[..., 1:] = pr.transpose(0, 1, 3, 4, 2, 5, 6)
        p_blk = p4.reshape(B, NQ * P, P3FREE2)
        return [
            {
                "f_in": np.ascontiguousarray(f_blk[c]),
                "p_in": np.ascontiguousarray(p_blk[c]),
            }
            for c in range(N_CORES)
        ]

    if os.environ.get("DYNF_KERNEL_VERSION", "4") == "3":
        # v3 fp16 layouts: f as (s, dh, dw | o, j, k28), p as (s, dh, dw | j, k28)
        patches = _im2col_batch(x)  # (B, H, W, 27)
        fr = f.reshape(B, N_ST, DH, DW, G, KK, CO)
        f3 = np.zeros((B, N_ST, DH, DW, CO, G, KP), np.float16)
        f3[..., 1:] = fr.transpose(0, 1, 2, 3, 6, 4, 5)
        f_blk = f3.reshape(B, N_ST * P, F3FREE)
        pr = patches.reshape(B, N_ST, DH, DW, G, KK)
        p3 = np.zeros((B, N_ST, DH, DW, G, KP), np.float16)
        p3[..., 1:] = pr
        p_blk = p3.reshape(B, N_ST * P, P3FREE)
        return [
            {
                "f_in": np.ascontiguousarray(f_blk[c]),
                "p_in": np.ascontiguousarray(p_blk[c]),
            }
            for c in range(N_CORES)
        ]

    if os.environ.get("DYNF_PATCH_MODE", "packed") == "expand":
        p_blk = _xpp_batch(x)  # (B, N_ST*P, 144)
    else:
        patches = _im2col_batch(x)  # (B, H, W, 27)
        # block to the supertile layout: (H, W, .) -> (n_st, dh, dw, g, .)
        # h = s*8 + dh ; w = dw*12 + g ; partition p = dh*16 + dw
        p_blk = patches.reshape(B, N_ST, DH, DW, G, KK).reshape(B, N_ST * P, PFREE)
    f_blk = f.reshape(B, N_ST * P, FFREE)  # pure reshape: row-major slabs
    return [
        {"f_in": np.ascontiguousarray(f_blk[c]), "p_in": np.ascontiguousarray(p_blk[c])}
        for c in range(N_CORES)
    ]


def kernel(x: np.ndarray, f: np.ndarray) -> np.ndarray:
    import concourse.bass_utils as bass_utils

    nc = _get_nc()  # before staging: a v2->v1 fallback switches p_in layout
    in_maps = prepare_in_maps(x, f)
    res = bass_utils.run_bass_kernel_spmd(nc, in_maps, core_ids=list(range(N_CORES)))
    global LAST_RESULTS
    LAST_RESULTS = res

    out = np.empty((B, H, W, CO), dtype=np.float32)
    for c in range(N_CORES):
        o = res.results[c]["o_out"]  # (N_ST*P, OFREE), f32 (or f16 if DYNF_OUT16)
        out[c] = o.reshape(H, W, CO).astype(np.float32)
    return out

